# revision 11
# baseline (speedup 1.0000x reference)
"""Bass/Trainium2 kernel for nn_BigramLanguageModel (8-layer GPT-ish, quirky
softmax-over-query-axis attention).

Sharding: 8 cores = DP2 (batch) x TP4.  Core c = 4*b + r handles batch b;
within a DP group of 4 cores: tokens are sharded (512/core) for residual /
LN / FFN (full width, no collective), heads are sharded (4/core) for
attention, with ReduceScatter(tokens) after proj and AllGather(n1^T)
before QKV; the lm_head is V-sharded (8000/core) after a final AllGather.

Precision: residual stream, LN, softmax normalizers, and all PSUM
accumulation in fp32; big streamed operands (weights, activations feeding
matmuls) in bf16; attention-proj in f32r.  LayerNorm gamma/beta are folded
into consumer weights on the host; bias paths exist behind flags (the
actual inputs have zero biases / unit gains, so they are skipped -- the
folding keeps results correct for any inputs).
"""
import sys

sys.path.insert(0, "/opt/trn_rl_repo")

import numpy as np
import ml_dtypes
import concourse.bacc as bacc
import concourse.mybir as mybir
import concourse.tile as tile
from concourse.bass_utils import run_bass_kernel_spmd
from concourse.masks import make_identity

B, T, V, D, H, L = 2, 2048, 32000, 1024, 16, 8
HS, F = 64, 4096
NC = 8
TPG = 4
TOK = T // TPG   # 512 tokens per core
NH = H // TPG    # 4 heads per core
VS = V // TPG    # 8000 vocab per core
DT = D // 128    # 8
TT = TOK // 128  # 4
TT_ALL = T // 128  # 16
FT = F // 128    # 32
FQ = FT // 4     # 8 hidden tiles per FFN quarter-pass
VC = 500
NVC = VS // VC   # 16
G4 = [[0, 1, 2, 3], [4, 5, 6, 7]]

f32 = mybir.dt.float32
f32r = mybir.dt.float32r
bf16 = mybir.dt.bfloat16
AL = mybir.AluOpType
AF = mybir.ActivationFunctionType
AX = mybir.AxisListType

_BUILD_CACHE = {}


def _ln_stats(nc, pool, x_ap, scratch, n_out_ap, dinv):
    """Free-axis LayerNorm: n_out = (x - mean) * rstd for a [128, width] tile."""
    s = pool.tile([128, 1], f32, tag="ln_s")
    ssq = pool.tile([128, 1], f32, tag="ln_ssq")
    nc.vector.tensor_reduce(s[:], x_ap, AX.X, AL.add)
    nc.vector.scalar_tensor_tensor(
        out=scratch, in0=x_ap, scalar=1.0, in1=x_ap,
        op0=AL.mult, op1=AL.mult, accum_out=ssq[:],
    )
    m = pool.tile([128, 1], f32, tag="ln_m")
    nc.vector.tensor_scalar_mul(m[:], s[:], dinv)
    var = pool.tile([128, 1], f32, tag="ln_var")
    nc.vector.tensor_scalar_mul(var[:], ssq[:], dinv)
    t1 = pool.tile([128, 1], f32, tag="ln_t1")
    nc.vector.tensor_mul(t1[:], m[:], m[:])
    nc.vector.tensor_sub(var[:], var[:], t1[:])
    nc.vector.tensor_scalar_add(var[:], var[:], 1e-5)
    sd = pool.tile([128, 1], f32, tag="ln_sd")
    nc.scalar.sqrt(sd[:], var[:])
    r = pool.tile([128, 1], f32, tag="ln_r")
    nc.vector.reciprocal(r[:], sd[:])
    nmr = pool.tile([128, 1], f32, tag="ln_nmr")
    nc.vector.tensor_mul(nmr[:], m[:], r[:])
    nc.vector.tensor_scalar_mul(nmr[:], nmr[:], -1.0)
    nc.scalar.activation(n_out_ap, x_ap, AF.Identity, bias=nmr[:], scale=r[:])


def _transpose_block(nc, tc, pool_src_ap, ident, out_tile, l_tag):
    """PE-transpose [128, TT, D]-style token-major tile into [128, DT, TOK]."""
    with tc.tile_pool(name=f"ps_tp_{l_tag}", bufs=3, space="PSUM") as ps_tp:
        for tt in range(TT):
            for dt in range(DT):
                tp = ps_tp.tile([128, 128], f32, tag="tp")
                nc.tensor.transpose(
                    tp[:], pool_src_ap[:, tt, dt * 128:(dt + 1) * 128], ident[:]
                )
                nc.scalar.copy(out_tile[:, dt, tt * 128:(tt + 1) * 128], tp[:])


def _build(flags):
    (fl_qkb, fl_vb, fl_b1, fl_g1, fl_b1f, fl_b2, fl_g2, fl_blm) = flags
    nc = bacc.Bacc("TRN2", target_bir_lowering=False, debug=False, num_devices=NC)

    x0_e = nc.declare_dram_parameter("x0", [TOK, D], f32, isOutput=False)
    wqkv_e = nc.declare_dram_parameter("wqkv", [L, D, 768], bf16, isOutput=False)
    wpr_e = nc.declare_dram_parameter("wpr", [L, 64, NH, D], bf16, isOutput=False)
    w1_e = nc.declare_dram_parameter("w1", [L, D, F], bf16, isOutput=False)
    w2_e = nc.declare_dram_parameter("w2", [L, F, D], bf16, isOutput=False)
    wlm_e = nc.declare_dram_parameter("wlm", [D, VS], bf16, isOutput=False)
    msk_e = nc.declare_dram_parameter("msk", [128, 128], f32, isOutput=False)
    if fl_qkb:
        qkb_e = nc.declare_dram_parameter("qkb", [L, 2, 256], f32, isOutput=False)
    if fl_vb:
        vb_e = nc.declare_dram_parameter("vb", [L, 1, 256], f32, isOutput=False)
    if fl_b1:
        b1r_e = nc.declare_dram_parameter("b1r", [L, 1, D], f32, isOutput=False)
    if fl_g1:
        g1r_e = nc.declare_dram_parameter("g1r", [L, 1, D], f32, isOutput=False)
    if fl_b1f:
        b1f_e = nc.declare_dram_parameter("b1f", [L, 128, FT], f32, isOutput=False)
    if fl_b2:
        b2r_e = nc.declare_dram_parameter("b2r", [L, 1, D], f32, isOutput=False)
    if fl_g2:
        g2r_e = nc.declare_dram_parameter("g2r", [L, 1, D], f32, isOutput=False)
    if fl_blm:
        blm_e = nc.declare_dram_parameter("blmv", [1, VS], f32, isOutput=False)

    lg_e = nc.declare_dram_parameter("logits", [T, VS], f32, isOutput=True)
    zs_e = nc.declare_dram_parameter("zsum", [T, 1], f32, isOutput=True)

    with tile.TileContext(nc) as tc:
        with (
            tc.tile_pool(name="const", bufs=1) as cpool,
            tc.tile_pool(name="small", bufs=2) as spool,
            tc.tile_pool(name="scr", bufs=1) as scrpool,
            tc.tile_pool(name="resid", bufs=1) as rpool,
            tc.tile_pool(name="big", bufs=1) as bpool,
            tc.tile_pool(name="attn", bufs=2) as apool,
            tc.tile_pool(name="wstream", bufs=1) as wpool1,
            tc.tile_pool(name="wstream3", bufs=4) as wpool3,
            tc.tile_pool(name="wstream2", bufs=2) as wpool2,
            tc.tile_pool(name="evict", bufs=2) as epool,
            tc.tile_pool(name="dram", bufs=2, space="DRAM") as dram,
        ):
            ident = cpool.tile([128, 128], f32)
            make_identity(nc, ident[:])
            msk = cpool.tile([128, 128], f32)
            nc.sync.dma_start(msk[:], msk_e[:])

            x = rpool.tile([128, TT, D], f32, tag="x", name="x_init")
            for tt in range(TT):
                nc.sync.dma_start(x[:, tt, :], x0_e[tt * 128:(tt + 1) * 128, :])

            for l in range(L):
                # ---------------- LN1 -> n1 ----------------
                n1 = rpool.tile([128, TT, D], f32, tag="n1", name=f"n1_{l}")
                scratch = scrpool.tile([128, D], f32, tag="scratch", name=f"sc1_{l}")
                for tt in range(TT):
                    _ln_stats(nc, spool, x[:, tt, :], scratch[:], n1[:, tt, :], 1.0 / D)

                # ---------------- transpose n1 -> n1T (bf16) -------------------
                n1T = bpool.tile([128, DT, TOK], bf16, tag="n1T", name=f"n1T_{l}")
                _transpose_block(nc, tc, n1, ident, n1T, f"t1_{l}")

                # ---------------- AG n1T -------------------
                agi = dram.tile([D, TOK], bf16, tag="agi", name=f"agi_{l}")
                ago = dram.tile([TPG * D, TOK], bf16, tag="ago", name=f"ago_{l}")
                for dt in range(DT):
                    nc.sync.dma_start(agi[dt * 128:(dt + 1) * 128, :], n1T[:, dt, :])
                nc.gpsimd.collective_compute(
                    "AllGather", AL.bypass, replica_groups=G4,
                    ins=[agi.opt()], outs=[ago.opt()],
                )
                nT = bpool.tile([128, DT, T], bf16, tag="nT", name=f"nT_{l}")
                for rr in range(TPG):
                    for dt in range(DT):
                        # rank rr's block holds its local tiles c=0..3 = global tiles 4c+rr
                        nc.sync.dma_start(
                            nT[:, dt, :].rearrange("p (c4 rr2 f) -> p c4 rr2 f", rr2=TPG, f=128)[:, :, rr, :],
                            ago[rr * D + dt * 128: rr * D + (dt + 1) * 128, :],
                        )
                # ---------------- QKV ----------------
                wqkv = wpool1.tile([128, DT, 768], bf16, tag="wqkv", name=f"wqkv_{l}")
                nc.sync.dma_start(
                    wqkv[:], wqkv_e[l].rearrange("(dt p) c -> p dt c", p=128)
                )
                if fl_qkb:
                    qkb = spool.tile([128, 4], f32, tag="qkb")
                    nc.sync.dma_start(
                        qkb[:], qkb_e[l].rearrange("two (pr p) -> p (two pr)", p=128)
                    )
                qT = bpool.tile([128, 2, T], bf16, tag="qT", name=f"qT_{l}")
                kT = bpool.tile([128, 2, T], bf16, tag="kT", name=f"kT_{l}")
                v_sb = bpool.tile([128, TT_ALL, 256], bf16, tag="v_sb", name=f"v_{l}")
                if fl_vb:
                    vbt = spool.tile([128, 256], f32, tag="vbt")
                    nc.sync.dma_start(vbt[:], vb_e[l].broadcast_to([128, 256]))
                with tc.tile_pool(name=f"ps_qkv_{l}", bufs=3, space="PSUM") as ps_qkv:
                    for p in range(2):
                        for c4 in range(4):
                            psq = ps_qkv.tile([128, 512], f32, tag="psq")
                            psk = ps_qkv.tile([128, 512], f32, tag="psk")
                            for dt in range(DT):
                                nc.tensor.matmul(
                                    psq[:], wqkv[:, dt, 128 * p:128 * (p + 1)],
                                    nT[:, dt, 512 * c4:512 * (c4 + 1)],
                                    start=(dt == 0), stop=(dt == DT - 1),
                                )
                                nc.tensor.matmul(
                                    psk[:], wqkv[:, dt, 256 + 128 * p:256 + 128 * (p + 1)],
                                    nT[:, dt, 512 * c4:512 * (c4 + 1)],
                                    start=(dt == 0), stop=(dt == DT - 1),
                                )
                            sl = (slice(None), p, slice(512 * c4, 512 * (c4 + 1)))
                            if fl_qkb:
                                nc.scalar.activation(qT[sl], psq[:], AF.Identity,
                                                     bias=qkb[:, p:p + 1])
                                nc.scalar.activation(kT[sl], psk[:], AF.Identity,
                                                     bias=qkb[:, 2 + p:3 + p])
                            else:
                                nc.vector.tensor_copy(qT[sl], psq[:])
                                nc.vector.tensor_copy(kT[sl], psk[:])
                        for t8 in range(2 * p * 4, 2 * p * 4 + 8):
                            psv = ps_qkv.tile([128, 256], f32, tag="psv", bufs=2)
                            for dt in range(DT):
                                nc.tensor.matmul(
                                    psv[:], nT[:, dt, 128 * t8:128 * (t8 + 1)],
                                    wqkv[:, dt, 512:768],
                                    start=(dt == 0), stop=(dt == DT - 1),
                                )
                            if fl_vb:
                                nc.vector.tensor_add(v_sb[:, t8, :], psv[:], vbt[:])
                            else:
                                nc.vector.tensor_copy(v_sb[:, t8, :], psv[:])

                # ---------------- attention ----------------
                # rows = keys s, cols = queries t; softmax over t per key s.
                oT = bpool.tile([64, NH, T], bf16, tag="oT", name=f"oT_{l}")
                for h in range(NH):
                    p, lo = h // 2, 64 * (h % 2)
                    with tc.tile_pool(name=f"ps_at_{l}_{h}", bufs=1, space="PSUM") as ps_at, \
                         tc.tile_pool(name=f"ps_sc_{l}_{h}", bufs=4, space="PSUM") as ps_sc:
                        o_ps = ps_at.tile([64, T], f32, tag="o_ps", name=f"ops_{l}_{h}")

                        def emit_scores(j):
                            base = 128 * j
                            E = apool.tile([128, 2048], bf16, tag="E",
                                           name=f"E_{l}_{h}_{j}")
                            zacc = spool.tile([128, 1], f32, tag="zacc")
                            pieces = [(base, 512 * (j // 4 + 1))]
                            while pieces[-1][1] < T:
                                pieces.append((pieces[-1][1], pieces[-1][1] + 512))
                            for pi, (a0, a1) in enumerate(pieces):
                                wid = a1 - a0
                                sc = ps_sc.tile([128, 512], f32, tag="sc")
                                nc.tensor.matmul(
                                    sc[:, :wid],
                                    kT[lo:lo + 64, p, base:base + 128],
                                    qT[lo:lo + 64, p, a0:a1],
                                    start=True, stop=True,
                                )
                                if pi == 0:
                                    nc.vector.tensor_add(sc[:, :128], sc[:, :128], msk[:])
                                zp = spool.tile([128, 1], f32, tag="zp")
                                nc.scalar.activation(
                                    E[:, a0 - base:a1 - base], sc[:, :wid], AF.Exp,
                                    accum_out=zp[:],
                                )
                                if pi == 0:
                                    nc.vector.tensor_copy(zacc[:], zp[:])
                                else:
                                    nc.vector.tensor_add(zacc[:], zacc[:], zp[:])
                            rz = spool.tile([128, 1], f32, tag="rz")
                            nc.vector.reciprocal(rz[:], zacc[:])
                            vt = spool.tile([128, 64], bf16, tag="vt", bufs=3)
                            nc.vector.tensor_scalar_mul(
                                vt[:], v_sb[:, j, 64 * h:64 * (h + 1)], rz[:]
                            )
                            return E, vt

                        def emit_o(j, E, vt):
                            base = 128 * j
                            for c in range(j // 4, 4):
                                a0 = max(512 * c, base)
                                a1 = 512 * (c + 1)
                                nc.tensor.matmul(
                                    o_ps[:, a0:a1],
                                    vt[:], E[:, a0 - base:a1 - base],
                                    start=(j == 0), stop=(j == min(4 * c + 3, TT_ALL - 1)),
                                    skip_group_check=True,
                                )

                        # software pipeline by one s-tile: scores_j ahead of o_{j-1}
                        prev = None
                        for j in range(TT_ALL):
                            cur = emit_scores(j)
                            if prev is not None:
                                emit_o(j - 1, *prev)
                            prev = cur
                        emit_o(TT_ALL - 1, *prev)
                        nc.scalar.copy(oT[:, h, :], o_ps[:])

                # ---------------- proj (token-major partial) + RS ----------------
                wpr = wpool1.tile([64, NH, D], bf16, tag="wpr", name=f"wpr_{l}")
                nc.sync.dma_start(wpr[:], wpr_e[l])
                rso_blks = []
                with tc.tile_pool(name=f"ps_pr_{l}", bufs=4, space="PSUM") as ps_pr:
                    for cch in range(4):
                        ppd = dram.tile([TOK, D], bf16, tag=f"ppd{cch}", name=f"ppd_{l}_{cch}")
                        for rr in range(TPG):
                            t8 = 4 * cch + rr
                            for dc in range(2):
                                psp = ps_pr.tile([128, 512], f32, tag="psp")
                                for h in range(NH):
                                    nc.tensor.matmul(
                                        psp[:],
                                        oT[:, h, 128 * t8:128 * (t8 + 1)],
                                        wpr[:, h, 512 * dc:512 * (dc + 1)],
                                        start=(h == 0), stop=(h == NH - 1),
                                    )
                                pp = epool.tile([128, 512], bf16, tag="pp")
                                nc.scalar.copy(pp[:], psp[:])
                                nc.sync.dma_start(
                                    ppd[128 * rr:128 * (rr + 1), 512 * dc:512 * (dc + 1)],
                                    pp[:],
                                )
                        rso = dram.tile([128, D], bf16, tag=f"rso{cch}", name=f"rso_{l}_{cch}")
                        nc.gpsimd.collective_compute(
                            "ReduceScatter", AL.add, replica_groups=G4,
                            ins=[ppd.opt()], outs=[rso.opt()],
                        )
                        rso_blks.append(rso)
                # ---------------- x' = n1 (*g1) + o (+bias1) ----------------
                xp = rpool.tile([128, TT, D], f32, tag="x", name=f"xp_{l}")
                if fl_g1:
                    g1t = spool.tile([128, D], f32, tag="g1t")
                    nc.sync.dma_start(g1t[:], g1r_e[l].broadcast_to([128, D]))
                if fl_b1:
                    b1t = spool.tile([128, D], f32, tag="b1t")
                    nc.sync.dma_start(b1t[:], b1r_e[l].broadcast_to([128, D]))
                for tt in range(TT):
                    ot = epool.tile([128, D], bf16, tag="ot")
                    nc.sync.dma_start(ot[:], rso_blks[tt][:])
                    if fl_g1:
                        tmp = epool.tile([128, D], f32, tag="rtmp")
                        nc.vector.tensor_mul(tmp[:], n1[:, tt, :], g1t[:])
                        nc.vector.tensor_add(xp[:, tt, :], tmp[:], ot[:])
                    else:
                        nc.vector.tensor_add(xp[:, tt, :], n1[:, tt, :], ot[:])
                    if fl_b1:
                        nc.vector.tensor_add(xp[:, tt, :], xp[:, tt, :], b1t[:])

                # ---------------- LN2 -> n2 -> n2T ----------------
                n2 = rpool.tile([128, TT, D], f32, tag="n1", name=f"n2_{l}")
                scratch2 = scrpool.tile([128, D], f32, tag="scratch", name=f"sc2_{l}")
                for tt in range(TT):
                    _ln_stats(nc, spool, xp[:, tt, :], scratch2[:], n2[:, tt, :], 1.0 / D)
                n2T = bpool.tile([128, DT, TOK], bf16, tag="n1T", name=f"n2T_{l}")
                _transpose_block(nc, tc, n2, ident, n2T, f"t2_{l}")

                # ---------------- FFN (quarter passes over hidden) ----------------
                if fl_b1f:
                    b1ft = spool.tile([128, FT], f32, tag="b1ft")
                    nc.sync.dma_start(b1ft[:], b1f_e[l])
                xpp = rpool.tile([128, TT, D], f32, tag="x", name=f"xpp_{l}")
                if fl_g2:
                    g2t = spool.tile([128, D], f32, tag="g1t")
                    nc.sync.dma_start(g2t[:], g2r_e[l].broadcast_to([128, D]))
                if fl_b2:
                    b2t = spool.tile([128, D], f32, tag="b1t")
                    nc.sync.dma_start(b2t[:], b2r_e[l].broadcast_to([128, D]))
                for q in range(4):
                    h1T = bpool.tile([128, FQ, TOK], bf16, tag="h1T", name=f"h1T_{l}_{q}")
                    with tc.tile_pool(name=f"ps_f1_{l}_{q}", bufs=4, space="PSUM") as ps_f1:
                        for hq in range(FQ):
                            hs = q * FQ + hq
                            w1t = wpool3.tile([128, DT, 128], bf16, tag="w1t")
                            nc.sync.dma_start(
                                w1t[:],
                                w1_e[l].rearrange("(dt p) f -> p dt f", p=128)[
                                    :, :, 128 * hs:128 * (hs + 1)
                                ],
                            )
                            psh = ps_f1.tile([128, 512], f32, tag="psh")
                            for dt in range(DT):
                                nc.tensor.matmul(
                                    psh[:], w1t[:, dt, :], n2T[:, dt, :],
                                    start=(dt == 0), stop=(dt == DT - 1),
                                )
                            if fl_b1f:
                                nc.scalar.activation(h1T[:, hq, :], psh[:], AF.Relu,
                                                     bias=b1ft[:, hs:hs + 1])
                            else:
                                nc.vector.tensor_relu(h1T[:, hq, :], psh[:])
                    with tc.tile_pool(name=f"ps_f2_{l}_{q}", bufs=1, space="PSUM") as ps_f2:
                        fps = [
                            ps_f2.tile([128, 512], f32, tag=f"fps{i}", name=f"fps_{l}_{q}_{i}")
                            for i in range(8)
                        ]
                        for hq in range(FQ):
                            hs = q * FQ + hq
                            w2t = wpool3.tile([128, D], bf16, tag="w2t")
                            nc.sync.dma_start(w2t[:], w2_e[l][128 * hs:128 * (hs + 1), :])
                            for tt in range(TT):
                                for dc in range(2):
                                    nc.tensor.matmul(
                                        fps[tt * 2 + dc][:],
                                        h1T[:, hq, 128 * tt:128 * (tt + 1)],
                                        w2t[:, 512 * dc:512 * (dc + 1)],
                                        start=(hq == 0), stop=(hq == FQ - 1),
                                    )
                        # xpp accumulates: q==0: n2(*g2) + f_q ; else xpp += f_q
                        for tt in range(TT):
                            for dc in range(2):
                                fsl = (slice(None), tt, slice(512 * dc, 512 * (dc + 1)))
                                if q == 0:
                                    if fl_g2:
                                        tmp2 = epool.tile([128, 512], f32, tag="rtmp2")
                                        nc.vector.tensor_mul(
                                            tmp2[:], n2[fsl], g2t[:, 512 * dc:512 * (dc + 1)]
                                        )
                                        nc.vector.tensor_add(xpp[fsl], tmp2[:], fps[tt * 2 + dc][:])
                                    else:
                                        nc.vector.tensor_add(xpp[fsl], n2[fsl], fps[tt * 2 + dc][:])
                                else:
                                    nc.vector.tensor_add(xpp[fsl], xpp[fsl], fps[tt * 2 + dc][:])
                                if q == 3 and fl_b2:
                                    nc.vector.tensor_add(
                                        xpp[fsl], xpp[fsl], b2t[:, 512 * dc:512 * (dc + 1)]
                                    )
                x = xpp

            # ---------------- final LN + AG + lm head ----------------
            nf = rpool.tile([128, TT, D], f32, tag="n1", name="nf")
            scratch3 = scrpool.tile([128, D], f32, tag="scratch", name="sc3")
            for tt in range(TT):
                _ln_stats(nc, spool, x[:, tt, :], scratch3[:], nf[:, tt, :], 1.0 / D)
            nfT = bpool.tile([128, DT, TOK], bf16, tag="n1T", name="nfT")
            _transpose_block(nc, tc, nf, ident, nfT, "tf")
            nTf = bpool.tile([128, DT, T], bf16, tag="nT", name="nTf")
            for cch in range(4):
                agi_f = dram.tile([D, 128], bf16, tag=f"agi{cch}", name=f"agi_f_{cch}")
                ago_f = dram.tile([TPG * D, 128], bf16, tag=f"ago{cch}", name=f"ago_f_{cch}")
                for dt in range(DT):
                    nc.sync.dma_start(agi_f[dt * 128:(dt + 1) * 128, :],
                                      nfT[:, dt, 128 * cch:128 * (cch + 1)])
                nc.gpsimd.collective_compute(
                    "AllGather", AL.bypass, replica_groups=G4,
                    ins=[agi_f.opt()], outs=[ago_f.opt()],
                )
                for rr in range(TPG):
                    t8 = 4 * cch + rr
                    for dt in range(DT):
                        nc.sync.dma_start(
                            nTf[:, dt, 128 * t8:128 * (t8 + 1)],
                            ago_f[rr * D + dt * 128: rr * D + (dt + 1) * 128, :],
                        )

            zac = bpool.tile([128, TT_ALL], f32, tag="zac", name="zac")
            with tc.tile_pool(name="ps_lm", bufs=4, space="PSUM") as ps_lm:
                for vc in range(NVC):
                    wlmt = wpool2.tile([128, DT, VC], bf16, tag="wlmt")
                    nc.sync.dma_start(
                        wlmt[:],
                        wlm_e.rearrange("(dt p) v -> p dt v", p=128)[
                            :, :, VC * vc:VC * (vc + 1)
                        ],
                    )
                    if fl_blm:
                        blmt_c = spool.tile([128, VC], f32, tag="blmt")
                        nc.sync.dma_start(
                            blmt_c[:],
                            blm_e[:, VC * vc:VC * (vc + 1)].broadcast_to([128, VC]),
                        )
                    for t8 in range(TT_ALL):
                        psz = ps_lm.tile([128, VC], f32, tag="psz")
                        for dt in range(DT):
                            nc.tensor.matmul(
                                psz[:], nTf[:, dt, 128 * t8:128 * (t8 + 1)],
                                wlmt[:, dt, :],
                                start=(dt == 0), stop=(dt == DT - 1),
                            )
                        lgt = epool.tile([128, VC], f32, tag="lgt")
                        if fl_blm:
                            nc.vector.tensor_add(lgt[:], psz[:], blmt_c[:])
                        else:
                            nc.vector.tensor_copy(lgt[:], psz[:])
                        nc.sync.dma_start(
                            lg_e[128 * t8:128 * (t8 + 1), VC * vc:VC * (vc + 1)], lgt[:]
                        )
                        esc = epool.tile([128, VC], f32, tag="esc")
                        zp2 = spool.tile([128, 1], f32, tag="zp2")
                        nc.scalar.activation(esc[:], lgt[:], AF.Exp, accum_out=zp2[:])
                        if vc == 0:
                            nc.vector.tensor_copy(zac[:, t8:t8 + 1], zp2[:])
                        else:
                            nc.vector.tensor_add(zac[:, t8:t8 + 1], zac[:, t8:t8 + 1], zp2[:])
            for t8 in range(TT_ALL):
                nc.sync.dma_start(zs_e[128 * t8:128 * (t8 + 1), :], zac[:, t8:t8 + 1])

    nc.compile()
    return nc


def kernel(**inputs):
    idx = np.asarray(inputs["idx"]).astype(np.int64)
    targets = np.asarray(inputs["targets"]).astype(np.int64)
    f = lambda k: np.asarray(inputs[k], dtype=np.float32)
    tok_emb, pos_emb = f("tok_emb"), f("pos_emb")
    wq, wk, wv = f("wq"), f("wk"), f("wv")
    wproj, bproj = f("wproj"), f("bproj")
    ln1_g, ln1_b, ln2_g, ln2_b = f("ln1_g"), f("ln1_b"), f("ln2_g"), f("ln2_b")
    w1, b1, w2, b2 = f("w1"), f("b1"), f("w2"), f("b2")
    lnf_g, lnf_b, wlm, blm = f("lnf_g"), f("lnf_b"), f("wlm"), f("blm")

    scale = D ** -0.5
    bf = ml_dtypes.bfloat16

    wq_gs = wq * ln1_g[:, None, :, None] * scale
    wk_g = wk * ln1_g[:, None, :, None]
    wv_g = wv * ln1_g[:, None, :, None]
    bq = np.einsum("ld,lhde->lhe", ln1_b, wq) * scale
    bk = np.einsum("ld,lhde->lhe", ln1_b, wk)
    bv = np.einsum("ld,lhde->lhe", ln1_b, wv)
    w1_g = w1 * ln2_g[:, :, None]
    b1f = b1 + np.einsum("ld,ldf->lf", ln2_b, w1)
    bias1 = ln1_b + bproj
    bias2 = ln2_b + b2
    wlm_g = lnf_g[:, None] * wlm
    blm_eff = blm + lnf_b @ wlm

    flags = (
        bool(np.any(bq) or np.any(bk)), bool(np.any(bv)), bool(np.any(bias1)),
        not bool(np.all(ln1_g == 1.0)), bool(np.any(b1f)), bool(np.any(bias2)),
        not bool(np.all(ln2_g == 1.0)), bool(np.any(blm_eff)),
    )
    if flags not in _BUILD_CACHE:
        _BUILD_CACHE[flags] = _build(flags)
    nc = _BUILD_CACHE[flags]

    x0 = tok_emb[idx] + pos_emb[None, :T]
    maskbias = np.where(
        np.arange(128)[None, :] >= np.arange(128)[:, None], 0.0, -60.0
    ).astype(np.float32)

    in_maps = []
    for c in range(NC):
        b, r = divmod(c, TPG)
        hh = slice(NH * r, NH * (r + 1))
        wqkv_c = np.concatenate(
            [
                wq_gs[:, hh].transpose(0, 2, 1, 3).reshape(L, D, NH * HS),
                wk_g[:, hh].transpose(0, 2, 1, 3).reshape(L, D, NH * HS),
                wv_g[:, hh].transpose(0, 2, 1, 3).reshape(L, D, NH * HS),
            ],
            axis=2,
        ).astype(bf)
        # wpr: [L, 64, NH, D] -- row e of head h = wproj[64h + e]
        wpr_c = np.ascontiguousarray(
            wproj[:, 256 * r:256 * (r + 1), :].reshape(L, NH, 64, D).transpose(0, 2, 1, 3)
        ).astype(bf)
        m = {
            "x0": np.ascontiguousarray(
                np.concatenate([
                    x0[b, 512 * cc + 128 * r: 512 * cc + 128 * (r + 1)]
                    for cc in range(4)
                ], axis=0)
            ),
            "wqkv": np.ascontiguousarray(wqkv_c),
            "wpr": wpr_c,
            "w1": np.ascontiguousarray(w1_g.astype(bf)),
            "w2": np.ascontiguousarray(w2.astype(bf)),
            "wlm": np.ascontiguousarray(wlm_g[:, VS * r:VS * (r + 1)].astype(bf)),
            "msk": maskbias,
        }
        if flags[0]:
            m["qkb"] = np.ascontiguousarray(
                np.stack([bq[:, hh].reshape(L, 256), bk[:, hh].reshape(L, 256)], axis=1)
            )
        if flags[1]:
            m["vb"] = np.ascontiguousarray(bv[:, hh].reshape(L, 1, 256))
        if flags[2]:
            m["b1r"] = np.ascontiguousarray(bias1[:, None, :])
        if flags[3]:
            m["g1r"] = np.ascontiguousarray(ln1_g[:, None, :])
        if flags[4]:
            m["b1f"] = np.ascontiguousarray(b1f.reshape(L, FT, 128).transpose(0, 2, 1))
        if flags[5]:
            m["b2r"] = np.ascontiguousarray(bias2[:, None, :])
        if flags[6]:
            m["g2r"] = np.ascontiguousarray(ln2_g[:, None, :])
        if flags[7]:
            m["blmv"] = np.ascontiguousarray(blm_eff[None, VS * r:VS * (r + 1)])
        in_maps.append(m)

    r_ = run_bass_kernel_spmd(nc, in_maps, list(range(NC)))
    res = r_.results

    logits = np.empty((B * T, V), dtype=np.float32)
    Z = np.zeros((B, T), dtype=np.float64)
    for c in range(NC):
        b, r = divmod(c, TPG)
        logits[b * T:(b + 1) * T, VS * r:VS * (r + 1)] = res[c]["logits"]
        Z[b] += res[c]["zsum"][:, 0].astype(np.float64)
    lse = np.log(Z).reshape(B * T)
    tgt = targets.reshape(-1)
    tgt_logit = logits[np.arange(B * T), tgt].astype(np.float64)
    loss = np.float32(np.mean(lse - tgt_logit))
    return logits, loss


# revision 12
# speedup vs baseline: 1.0170x; 1.0170x over previous
"""Bass/Trainium2 kernel for nn_BigramLanguageModel (8-layer GPT-ish, quirky
softmax-over-query-axis attention).

Sharding: 8 cores = DP2 (batch) x TP4.  Core c = 4*b + r handles batch b;
within a DP group of 4 cores: tokens are sharded (512/core) for residual /
LN / FFN (full width, no collective), heads are sharded (4/core) for
attention, with ReduceScatter(tokens) after proj and AllGather(n1^T)
before QKV; the lm_head is V-sharded (8000/core) after a final AllGather.

Precision: residual stream, LN, softmax normalizers, and all PSUM
accumulation in fp32; big streamed operands (weights, activations feeding
matmuls) in bf16; attention-proj in f32r.  LayerNorm gamma/beta are folded
into consumer weights on the host; bias paths exist behind flags (the
actual inputs have zero biases / unit gains, so they are skipped -- the
folding keeps results correct for any inputs).
"""
import sys

sys.path.insert(0, "/opt/trn_rl_repo")

import numpy as np
import ml_dtypes
import concourse.bacc as bacc
import concourse.mybir as mybir
import concourse.tile as tile
from concourse.bass_utils import run_bass_kernel_spmd
from concourse.masks import make_identity

B, T, V, D, H, L = 2, 2048, 32000, 1024, 16, 8
HS, F = 64, 4096
NC = 8
TPG = 4
TOK = T // TPG   # 512 tokens per core
NH = H // TPG    # 4 heads per core
VS = V // TPG    # 8000 vocab per core
DT = D // 128    # 8
TT = TOK // 128  # 4
TT_ALL = T // 128  # 16
FT = F // 128    # 32
FQ = FT // 4     # 8 hidden tiles per FFN quarter-pass
VC = 500
NVC = VS // VC   # 16
G4 = [[0, 1, 2, 3], [4, 5, 6, 7]]

f32 = mybir.dt.float32
f32r = mybir.dt.float32r
bf16 = mybir.dt.bfloat16
AL = mybir.AluOpType
AF = mybir.ActivationFunctionType
AX = mybir.AxisListType

_BUILD_CACHE = {}


def _ln_stats(nc, pool, x_ap, scratch, n_out_ap, dinv):
    """Free-axis LayerNorm: n_out = (x - mean) * rstd for a [128, width] tile."""
    s = pool.tile([128, 1], f32, tag="ln_s")
    ssq = pool.tile([128, 1], f32, tag="ln_ssq")
    nc.vector.tensor_reduce(s[:], x_ap, AX.X, AL.add)
    nc.vector.scalar_tensor_tensor(
        out=scratch, in0=x_ap, scalar=1.0, in1=x_ap,
        op0=AL.mult, op1=AL.mult, accum_out=ssq[:],
    )
    m = pool.tile([128, 1], f32, tag="ln_m")
    nc.vector.tensor_scalar_mul(m[:], s[:], dinv)
    var = pool.tile([128, 1], f32, tag="ln_var")
    nc.vector.tensor_scalar_mul(var[:], ssq[:], dinv)
    t1 = pool.tile([128, 1], f32, tag="ln_t1")
    nc.vector.tensor_mul(t1[:], m[:], m[:])
    nc.vector.tensor_sub(var[:], var[:], t1[:])
    nc.vector.tensor_scalar_add(var[:], var[:], 1e-5)
    sd = pool.tile([128, 1], f32, tag="ln_sd")
    nc.scalar.sqrt(sd[:], var[:])
    r = pool.tile([128, 1], f32, tag="ln_r")
    nc.vector.reciprocal(r[:], sd[:])
    nmr = pool.tile([128, 1], f32, tag="ln_nmr")
    nc.vector.tensor_mul(nmr[:], m[:], r[:])
    nc.vector.tensor_scalar_mul(nmr[:], nmr[:], -1.0)
    nc.scalar.activation(n_out_ap, x_ap, AF.Identity, bias=nmr[:], scale=r[:])


def _transpose_block(nc, tc, pool_src_ap, ident, out_tile, l_tag):
    """PE-transpose [128, TT, D]-style token-major tile into [128, DT, TOK]."""
    with tc.tile_pool(name=f"ps_tp_{l_tag}", bufs=3, space="PSUM") as ps_tp:
        for tt in range(TT):
            for dt in range(DT):
                tp = ps_tp.tile([128, 128], f32, tag="tp")
                nc.tensor.transpose(
                    tp[:], pool_src_ap[:, tt, dt * 128:(dt + 1) * 128], ident[:]
                )
                nc.scalar.copy(out_tile[:, dt, tt * 128:(tt + 1) * 128], tp[:])


def _build(flags):
    (fl_qkb, fl_vb, fl_b1, fl_g1, fl_b1f, fl_b2, fl_g2, fl_blm) = flags
    nc = bacc.Bacc("TRN2", target_bir_lowering=False, debug=False, num_devices=NC)

    x0_e = nc.declare_dram_parameter("x0", [TOK, D], f32, isOutput=False)
    wqkv_e = nc.declare_dram_parameter("wqkv", [L, D, 768], bf16, isOutput=False)
    wpr_e = nc.declare_dram_parameter("wpr", [L, 64, NH, D], bf16, isOutput=False)
    w1_e = nc.declare_dram_parameter("w1", [L, D, F], bf16, isOutput=False)
    w2_e = nc.declare_dram_parameter("w2", [L, F, D], bf16, isOutput=False)
    wlm_e = nc.declare_dram_parameter("wlm", [D, VS], bf16, isOutput=False)
    msk_e = nc.declare_dram_parameter("msk", [128, 128], f32, isOutput=False)
    if fl_qkb:
        qkb_e = nc.declare_dram_parameter("qkb", [L, 2, 256], f32, isOutput=False)
    if fl_vb:
        vb_e = nc.declare_dram_parameter("vb", [L, 1, 256], f32, isOutput=False)
    if fl_b1:
        b1r_e = nc.declare_dram_parameter("b1r", [L, 1, D], f32, isOutput=False)
    if fl_g1:
        g1r_e = nc.declare_dram_parameter("g1r", [L, 1, D], f32, isOutput=False)
    if fl_b1f:
        b1f_e = nc.declare_dram_parameter("b1f", [L, 128, FT], f32, isOutput=False)
    if fl_b2:
        b2r_e = nc.declare_dram_parameter("b2r", [L, 1, D], f32, isOutput=False)
    if fl_g2:
        g2r_e = nc.declare_dram_parameter("g2r", [L, 1, D], f32, isOutput=False)
    if fl_blm:
        blm_e = nc.declare_dram_parameter("blmv", [1, VS], f32, isOutput=False)

    lg_e = nc.declare_dram_parameter("logits", [T, VS], f32, isOutput=True)
    zs_e = nc.declare_dram_parameter("zsum", [T, 1], f32, isOutput=True)

    with tile.TileContext(nc) as tc:
        with (
            tc.tile_pool(name="const", bufs=1) as cpool,
            tc.tile_pool(name="small", bufs=2) as spool,
            tc.tile_pool(name="scr", bufs=1) as scrpool,
            tc.tile_pool(name="resid", bufs=1) as rpool,
            tc.tile_pool(name="big", bufs=1) as bpool,
            tc.tile_pool(name="attn", bufs=2) as apool,
            tc.tile_pool(name="wstream", bufs=1) as wpool1,
            tc.tile_pool(name="wstream3", bufs=3) as wpool3,
            tc.tile_pool(name="wstream2", bufs=2) as wpool2,
            tc.tile_pool(name="evict", bufs=2) as epool,
            tc.tile_pool(name="dram", bufs=2, space="DRAM") as dram,
        ):
            ident = cpool.tile([128, 128], f32)
            make_identity(nc, ident[:])
            msk = cpool.tile([128, 128], f32)
            nc.sync.dma_start(msk[:], msk_e[:])

            x = rpool.tile([128, TT, D], f32, tag="x", name="x_init")
            for tt in range(TT):
                nc.sync.dma_start(x[:, tt, :], x0_e[tt * 128:(tt + 1) * 128, :])

            for l in range(L):
                # ---------------- LN1 -> n1 ----------------
                n1 = rpool.tile([128, TT, D], f32, tag="n1", name=f"n1_{l}")
                scratch = scrpool.tile([128, D], f32, tag="scratch", name=f"sc1_{l}")
                for tt in range(TT):
                    _ln_stats(nc, spool, x[:, tt, :], scratch[:], n1[:, tt, :], 1.0 / D)

                # ---------------- transpose n1 -> n1T (bf16) -------------------
                n1T = bpool.tile([128, DT, TOK], bf16, tag="n1T", name=f"n1T_{l}")
                _transpose_block(nc, tc, n1, ident, n1T, f"t1_{l}")

                # ---------------- AG n1T -------------------
                agi = dram.tile([D, TOK], bf16, tag="agi", name=f"agi_{l}")
                ago = dram.tile([TPG * D, TOK], bf16, tag="ago", name=f"ago_{l}")
                for dt in range(DT):
                    nc.sync.dma_start(agi[dt * 128:(dt + 1) * 128, :], n1T[:, dt, :])
                nc.gpsimd.collective_compute(
                    "AllGather", AL.bypass, replica_groups=G4,
                    ins=[agi.opt()], outs=[ago.opt()],
                )
                nT = bpool.tile([128, DT, T], bf16, tag="nT", name=f"nT_{l}")
                for rr in range(TPG):
                    for dt in range(DT):
                        # rank rr's block holds its local tiles c=0..3 = global tiles 4c+rr
                        nc.sync.dma_start(
                            nT[:, dt, :].rearrange("p (c4 rr2 f) -> p c4 rr2 f", rr2=TPG, f=128)[:, :, rr, :],
                            ago[rr * D + dt * 128: rr * D + (dt + 1) * 128, :],
                        )
                # ---------------- QKV ----------------
                wqkv = wpool1.tile([128, DT, 768], bf16, tag="wqkv", name=f"wqkv_{l}")
                nc.sync.dma_start(
                    wqkv[:], wqkv_e[l].rearrange("(dt p) c -> p dt c", p=128)
                )
                if fl_qkb:
                    qkb = spool.tile([128, 4], f32, tag="qkb")
                    nc.sync.dma_start(
                        qkb[:], qkb_e[l].rearrange("two (pr p) -> p (two pr)", p=128)
                    )
                qT = bpool.tile([128, 2, T], bf16, tag="qT", name=f"qT_{l}")
                kT = bpool.tile([128, 2, T], bf16, tag="kT", name=f"kT_{l}")
                v_sb = bpool.tile([128, TT_ALL, 256], bf16, tag="v_sb", name=f"v_{l}")
                if fl_vb:
                    vbt = spool.tile([128, 256], f32, tag="vbt")
                    nc.sync.dma_start(vbt[:], vb_e[l].broadcast_to([128, 256]))
                with tc.tile_pool(name=f"ps_qkv_{l}", bufs=3, space="PSUM") as ps_qkv:
                    for p in range(2):
                        for c4 in range(4):
                            psq = ps_qkv.tile([128, 512], f32, tag="psq")
                            psk = ps_qkv.tile([128, 512], f32, tag="psk")
                            for dt in range(DT):
                                nc.tensor.matmul(
                                    psq[:], wqkv[:, dt, 128 * p:128 * (p + 1)],
                                    nT[:, dt, 512 * c4:512 * (c4 + 1)],
                                    start=(dt == 0), stop=(dt == DT - 1),
                                )
                                nc.tensor.matmul(
                                    psk[:], wqkv[:, dt, 256 + 128 * p:256 + 128 * (p + 1)],
                                    nT[:, dt, 512 * c4:512 * (c4 + 1)],
                                    start=(dt == 0), stop=(dt == DT - 1),
                                )
                            sl = (slice(None), p, slice(512 * c4, 512 * (c4 + 1)))
                            if fl_qkb:
                                nc.scalar.activation(qT[sl], psq[:], AF.Identity,
                                                     bias=qkb[:, p:p + 1])
                                nc.scalar.activation(kT[sl], psk[:], AF.Identity,
                                                     bias=qkb[:, 2 + p:3 + p])
                            else:
                                nc.vector.tensor_copy(qT[sl], psq[:])
                                nc.vector.tensor_copy(kT[sl], psk[:])
                        for t8 in range(2 * p * 4, 2 * p * 4 + 8):
                            psv = ps_qkv.tile([128, 256], f32, tag="psv", bufs=2)
                            for dt in range(DT):
                                nc.tensor.matmul(
                                    psv[:], nT[:, dt, 128 * t8:128 * (t8 + 1)],
                                    wqkv[:, dt, 512:768],
                                    start=(dt == 0), stop=(dt == DT - 1),
                                )
                            if fl_vb:
                                nc.vector.tensor_add(v_sb[:, t8, :], psv[:], vbt[:])
                            else:
                                nc.vector.tensor_copy(v_sb[:, t8, :], psv[:])

                # ---------------- attention ----------------
                # rows = keys s, cols = queries t; softmax over t per key s.
                oT = bpool.tile([64, NH, T], bf16, tag="oT", name=f"oT_{l}")
                for h in range(NH):
                    p, lo = h // 2, 64 * (h % 2)
                    with tc.tile_pool(name=f"ps_at_{l}_{h}", bufs=1, space="PSUM") as ps_at, \
                         tc.tile_pool(name=f"ps_sc_{l}_{h}", bufs=4, space="PSUM") as ps_sc:
                        o_ps = ps_at.tile([64, T], f32, tag="o_ps", name=f"ops_{l}_{h}")

                        def emit_scores(j):
                            base = 128 * j
                            E = apool.tile([128, 2048], bf16, tag="E",
                                           name=f"E_{l}_{h}_{j}")
                            zacc = spool.tile([128, 1], f32, tag="zacc")
                            pieces = [(base, 512 * (j // 4 + 1))]
                            while pieces[-1][1] < T:
                                pieces.append((pieces[-1][1], pieces[-1][1] + 512))
                            for pi, (a0, a1) in enumerate(pieces):
                                wid = a1 - a0
                                sc = ps_sc.tile([128, 512], f32, tag="sc")
                                nc.tensor.matmul(
                                    sc[:, :wid],
                                    kT[lo:lo + 64, p, base:base + 128],
                                    qT[lo:lo + 64, p, a0:a1],
                                    start=True, stop=True,
                                )
                                if pi == 0:
                                    nc.vector.tensor_add(sc[:, :128], sc[:, :128], msk[:])
                                zp = spool.tile([128, 1], f32, tag="zp")
                                nc.scalar.activation(
                                    E[:, a0 - base:a1 - base], sc[:, :wid], AF.Exp,
                                    accum_out=zp[:],
                                )
                                if pi == 0:
                                    nc.vector.tensor_copy(zacc[:], zp[:])
                                else:
                                    nc.vector.tensor_add(zacc[:], zacc[:], zp[:])
                            rz = spool.tile([128, 1], f32, tag="rz")
                            nc.vector.reciprocal(rz[:], zacc[:])
                            vt = spool.tile([128, 64], bf16, tag="vt", bufs=3)
                            nc.vector.tensor_scalar_mul(
                                vt[:], v_sb[:, j, 64 * h:64 * (h + 1)], rz[:]
                            )
                            return E, vt

                        def emit_o(j, E, vt):
                            base = 128 * j
                            for c in range(j // 4, 4):
                                a0 = max(512 * c, base)
                                a1 = 512 * (c + 1)
                                nc.tensor.matmul(
                                    o_ps[:, a0:a1],
                                    vt[:], E[:, a0 - base:a1 - base],
                                    start=(j == 0), stop=(j == min(4 * c + 3, TT_ALL - 1)),
                                    skip_group_check=True,
                                )

                        # software pipeline by one s-tile: scores_j ahead of o_{j-1}
                        prev = None
                        for j in range(TT_ALL):
                            cur = emit_scores(j)
                            if prev is not None:
                                emit_o(j - 1, *prev)
                            prev = cur
                        emit_o(TT_ALL - 1, *prev)
                        nc.scalar.copy(oT[:, h, :], o_ps[:])

                # ---------------- proj (token-major partial) + RS ----------------
                wpr = wpool1.tile([64, NH, D], bf16, tag="wpr", name=f"wpr_{l}")
                nc.sync.dma_start(wpr[:], wpr_e[l])
                rso_blks = []
                with tc.tile_pool(name=f"ps_pr_{l}", bufs=4, space="PSUM") as ps_pr:
                    for cch in range(4):
                        ppd = dram.tile([TOK, D], bf16, tag=f"ppd{cch}", name=f"ppd_{l}_{cch}")
                        for rr in range(TPG):
                            t8 = 4 * cch + rr
                            for dc in range(2):
                                psp = ps_pr.tile([128, 512], f32, tag="psp")
                                for h in range(NH):
                                    nc.tensor.matmul(
                                        psp[:],
                                        oT[:, h, 128 * t8:128 * (t8 + 1)],
                                        wpr[:, h, 512 * dc:512 * (dc + 1)],
                                        start=(h == 0), stop=(h == NH - 1),
                                    )
                                pp = epool.tile([128, 512], bf16, tag="pp")
                                nc.scalar.copy(pp[:], psp[:])
                                nc.sync.dma_start(
                                    ppd[128 * rr:128 * (rr + 1), 512 * dc:512 * (dc + 1)],
                                    pp[:],
                                )
                        rso = dram.tile([128, D], bf16, tag=f"rso{cch}", name=f"rso_{l}_{cch}")
                        nc.gpsimd.collective_compute(
                            "ReduceScatter", AL.add, replica_groups=G4,
                            ins=[ppd.opt()], outs=[rso.opt()],
                        )
                        rso_blks.append(rso)
                # ---------------- x' = n1 (*g1) + o (+bias1) ----------------
                xp = rpool.tile([128, TT, D], f32, tag="x", name=f"xp_{l}")
                if fl_g1:
                    g1t = spool.tile([128, D], f32, tag="g1t")
                    nc.sync.dma_start(g1t[:], g1r_e[l].broadcast_to([128, D]))
                if fl_b1:
                    b1t = spool.tile([128, D], f32, tag="b1t")
                    nc.sync.dma_start(b1t[:], b1r_e[l].broadcast_to([128, D]))
                for tt in range(TT):
                    ot = epool.tile([128, D], bf16, tag="ot")
                    nc.sync.dma_start(ot[:], rso_blks[tt][:])
                    if fl_g1:
                        tmp = epool.tile([128, D], f32, tag="rtmp")
                        nc.vector.tensor_mul(tmp[:], n1[:, tt, :], g1t[:])
                        nc.vector.tensor_add(xp[:, tt, :], tmp[:], ot[:])
                    else:
                        nc.vector.tensor_add(xp[:, tt, :], n1[:, tt, :], ot[:])
                    if fl_b1:
                        nc.vector.tensor_add(xp[:, tt, :], xp[:, tt, :], b1t[:])

                # ---------------- LN2 -> n2 -> n2T ----------------
                n2 = rpool.tile([128, TT, D], f32, tag="n1", name=f"n2_{l}")
                scratch2 = scrpool.tile([128, D], f32, tag="scratch", name=f"sc2_{l}")
                for tt in range(TT):
                    _ln_stats(nc, spool, xp[:, tt, :], scratch2[:], n2[:, tt, :], 1.0 / D)
                n2T = bpool.tile([128, DT, TOK], bf16, tag="n1T", name=f"n2T_{l}")
                _transpose_block(nc, tc, n2, ident, n2T, f"t2_{l}")

                # ---------------- FFN (quarter passes over hidden) ----------------
                if fl_b1f:
                    b1ft = spool.tile([128, FT], f32, tag="b1ft")
                    nc.sync.dma_start(b1ft[:], b1f_e[l])
                xpp = rpool.tile([128, TT, D], f32, tag="x", name=f"xpp_{l}")
                if fl_g2:
                    g2t = spool.tile([128, D], f32, tag="g1t")
                    nc.sync.dma_start(g2t[:], g2r_e[l].broadcast_to([128, D]))
                if fl_b2:
                    b2t = spool.tile([128, D], f32, tag="b1t")
                    nc.sync.dma_start(b2t[:], b2r_e[l].broadcast_to([128, D]))
                for q in range(4):
                    h1T = bpool.tile([128, FQ, TOK], bf16, tag="h1T", name=f"h1T_{l}_{q}")
                    with tc.tile_pool(name=f"ps_f1_{l}_{q}", bufs=4, space="PSUM") as ps_f1:
                        for hq in range(FQ):
                            hs = q * FQ + hq
                            w1t = wpool3.tile([128, DT, 128], bf16, tag="w1t")
                            nc.sync.dma_start(
                                w1t[:],
                                w1_e[l].rearrange("(dt p) f -> p dt f", p=128)[
                                    :, :, 128 * hs:128 * (hs + 1)
                                ],
                            )
                            psh = ps_f1.tile([128, 512], f32, tag="psh")
                            for dt in range(DT):
                                nc.tensor.matmul(
                                    psh[:], w1t[:, dt, :], n2T[:, dt, :],
                                    start=(dt == 0), stop=(dt == DT - 1),
                                )
                            if fl_b1f:
                                nc.scalar.activation(h1T[:, hq, :], psh[:], AF.Relu,
                                                     bias=b1ft[:, hs:hs + 1])
                            else:
                                nc.vector.tensor_relu(h1T[:, hq, :], psh[:])
                    with tc.tile_pool(name=f"ps_f2_{l}_{q}", bufs=1, space="PSUM") as ps_f2:
                        fps = [
                            ps_f2.tile([128, 512], f32, tag=f"fps{i}", name=f"fps_{l}_{q}_{i}")
                            for i in range(8)
                        ]
                        for hq in range(FQ):
                            hs = q * FQ + hq
                            w2t = wpool3.tile([128, D], bf16, tag="w2t")
                            nc.sync.dma_start(w2t[:], w2_e[l][128 * hs:128 * (hs + 1), :])
                            for tt in range(TT):
                                for dc in range(2):
                                    nc.tensor.matmul(
                                        fps[tt * 2 + dc][:],
                                        h1T[:, hq, 128 * tt:128 * (tt + 1)],
                                        w2t[:, 512 * dc:512 * (dc + 1)],
                                        start=(hq == 0), stop=(hq == FQ - 1),
                                    )
                        # xpp accumulates: q==0: n2(*g2) + f_q ; else xpp += f_q
                        for tt in range(TT):
                            for dc in range(2):
                                fsl = (slice(None), tt, slice(512 * dc, 512 * (dc + 1)))
                                if q == 0:
                                    if fl_g2:
                                        tmp2 = epool.tile([128, 512], f32, tag="rtmp2")
                                        nc.vector.tensor_mul(
                                            tmp2[:], n2[fsl], g2t[:, 512 * dc:512 * (dc + 1)]
                                        )
                                        nc.vector.tensor_add(xpp[fsl], tmp2[:], fps[tt * 2 + dc][:])
                                    else:
                                        nc.vector.tensor_add(xpp[fsl], n2[fsl], fps[tt * 2 + dc][:])
                                else:
                                    nc.vector.tensor_add(xpp[fsl], xpp[fsl], fps[tt * 2 + dc][:])
                                if q == 3 and fl_b2:
                                    nc.vector.tensor_add(
                                        xpp[fsl], xpp[fsl], b2t[:, 512 * dc:512 * (dc + 1)]
                                    )
                x = xpp

            # ---------------- final LN + AG + lm head ----------------
            nf = rpool.tile([128, TT, D], f32, tag="n1", name="nf")
            scratch3 = scrpool.tile([128, D], f32, tag="scratch", name="sc3")
            for tt in range(TT):
                _ln_stats(nc, spool, x[:, tt, :], scratch3[:], nf[:, tt, :], 1.0 / D)
            nfT = bpool.tile([128, DT, TOK], bf16, tag="n1T", name="nfT")
            _transpose_block(nc, tc, nf, ident, nfT, "tf")
            nTf = bpool.tile([128, DT, T], bf16, tag="nT", name="nTf")
            for cch in range(4):
                agi_f = dram.tile([D, 128], bf16, tag=f"agi{cch}", name=f"agi_f_{cch}")
                ago_f = dram.tile([TPG * D, 128], bf16, tag=f"ago{cch}", name=f"ago_f_{cch}")
                for dt in range(DT):
                    nc.sync.dma_start(agi_f[dt * 128:(dt + 1) * 128, :],
                                      nfT[:, dt, 128 * cch:128 * (cch + 1)])
                nc.gpsimd.collective_compute(
                    "AllGather", AL.bypass, replica_groups=G4,
                    ins=[agi_f.opt()], outs=[ago_f.opt()],
                )
                for rr in range(TPG):
                    t8 = 4 * cch + rr
                    for dt in range(DT):
                        nc.sync.dma_start(
                            nTf[:, dt, 128 * t8:128 * (t8 + 1)],
                            ago_f[rr * D + dt * 128: rr * D + (dt + 1) * 128, :],
                        )

            zac = bpool.tile([128, TT_ALL], f32, tag="zac", name="zac")
            with tc.tile_pool(name="ps_lm", bufs=4, space="PSUM") as ps_lm:
                for vc in range(NVC):
                    wlmt = wpool2.tile([128, DT, VC], bf16, tag="wlmt")
                    nc.sync.dma_start(
                        wlmt[:],
                        wlm_e.rearrange("(dt p) v -> p dt v", p=128)[
                            :, :, VC * vc:VC * (vc + 1)
                        ],
                    )
                    if fl_blm:
                        blmt_c = spool.tile([128, VC], f32, tag="blmt")
                        nc.sync.dma_start(
                            blmt_c[:],
                            blm_e[:, VC * vc:VC * (vc + 1)].broadcast_to([128, VC]),
                        )
                    for t8 in range(TT_ALL):
                        psz = ps_lm.tile([128, VC], f32, tag="psz")
                        for dt in range(DT):
                            nc.tensor.matmul(
                                psz[:], nTf[:, dt, 128 * t8:128 * (t8 + 1)],
                                wlmt[:, dt, :],
                                start=(dt == 0), stop=(dt == DT - 1),
                            )
                        lgt = epool.tile([128, VC], f32, tag="lgt")
                        if fl_blm:
                            nc.vector.tensor_add(lgt[:], psz[:], blmt_c[:])
                        else:
                            nc.vector.tensor_copy(lgt[:], psz[:])
                        nc.sync.dma_start(
                            lg_e[128 * t8:128 * (t8 + 1), VC * vc:VC * (vc + 1)], lgt[:]
                        )
                        esc = epool.tile([128, VC], f32, tag="esc")
                        zp2 = spool.tile([128, 1], f32, tag="zp2")
                        nc.scalar.activation(esc[:], lgt[:], AF.Exp, accum_out=zp2[:])
                        if vc == 0:
                            nc.vector.tensor_copy(zac[:, t8:t8 + 1], zp2[:])
                        else:
                            nc.vector.tensor_add(zac[:, t8:t8 + 1], zac[:, t8:t8 + 1], zp2[:])
            for t8 in range(TT_ALL):
                nc.sync.dma_start(zs_e[128 * t8:128 * (t8 + 1), :], zac[:, t8:t8 + 1])

    nc.compile()
    return nc


def kernel(**inputs):
    idx = np.asarray(inputs["idx"]).astype(np.int64)
    targets = np.asarray(inputs["targets"]).astype(np.int64)
    f = lambda k: np.asarray(inputs[k], dtype=np.float32)
    tok_emb, pos_emb = f("tok_emb"), f("pos_emb")
    wq, wk, wv = f("wq"), f("wk"), f("wv")
    wproj, bproj = f("wproj"), f("bproj")
    ln1_g, ln1_b, ln2_g, ln2_b = f("ln1_g"), f("ln1_b"), f("ln2_g"), f("ln2_b")
    w1, b1, w2, b2 = f("w1"), f("b1"), f("w2"), f("b2")
    lnf_g, lnf_b, wlm, blm = f("lnf_g"), f("lnf_b"), f("wlm"), f("blm")

    scale = D ** -0.5
    bf = ml_dtypes.bfloat16

    wq_gs = wq * ln1_g[:, None, :, None] * scale
    wk_g = wk * ln1_g[:, None, :, None]
    wv_g = wv * ln1_g[:, None, :, None]
    bq = np.einsum("ld,lhde->lhe", ln1_b, wq) * scale
    bk = np.einsum("ld,lhde->lhe", ln1_b, wk)
    bv = np.einsum("ld,lhde->lhe", ln1_b, wv)
    w1_g = w1 * ln2_g[:, :, None]
    b1f = b1 + np.einsum("ld,ldf->lf", ln2_b, w1)
    bias1 = ln1_b + bproj
    bias2 = ln2_b + b2
    wlm_g = lnf_g[:, None] * wlm
    blm_eff = blm + lnf_b @ wlm

    flags = (
        bool(np.any(bq) or np.any(bk)), bool(np.any(bv)), bool(np.any(bias1)),
        not bool(np.all(ln1_g == 1.0)), bool(np.any(b1f)), bool(np.any(bias2)),
        not bool(np.all(ln2_g == 1.0)), bool(np.any(blm_eff)),
    )
    if flags not in _BUILD_CACHE:
        _BUILD_CACHE[flags] = _build(flags)
    nc = _BUILD_CACHE[flags]

    x0 = tok_emb[idx] + pos_emb[None, :T]
    maskbias = np.where(
        np.arange(128)[None, :] >= np.arange(128)[:, None], 0.0, -60.0
    ).astype(np.float32)

    in_maps = []
    for c in range(NC):
        b, r = divmod(c, TPG)
        hh = slice(NH * r, NH * (r + 1))
        wqkv_c = np.concatenate(
            [
                wq_gs[:, hh].transpose(0, 2, 1, 3).reshape(L, D, NH * HS),
                wk_g[:, hh].transpose(0, 2, 1, 3).reshape(L, D, NH * HS),
                wv_g[:, hh].transpose(0, 2, 1, 3).reshape(L, D, NH * HS),
            ],
            axis=2,
        ).astype(bf)
        # wpr: [L, 64, NH, D] -- row e of head h = wproj[64h + e]
        wpr_c = np.ascontiguousarray(
            wproj[:, 256 * r:256 * (r + 1), :].reshape(L, NH, 64, D).transpose(0, 2, 1, 3)
        ).astype(bf)
        m = {
            "x0": np.ascontiguousarray(
                np.concatenate([
                    x0[b, 512 * cc + 128 * r: 512 * cc + 128 * (r + 1)]
                    for cc in range(4)
                ], axis=0)
            ),
            "wqkv": np.ascontiguousarray(wqkv_c),
            "wpr": wpr_c,
            "w1": np.ascontiguousarray(w1_g.astype(bf)),
            "w2": np.ascontiguousarray(w2.astype(bf)),
            "wlm": np.ascontiguousarray(wlm_g[:, VS * r:VS * (r + 1)].astype(bf)),
            "msk": maskbias,
        }
        if flags[0]:
            m["qkb"] = np.ascontiguousarray(
                np.stack([bq[:, hh].reshape(L, 256), bk[:, hh].reshape(L, 256)], axis=1)
            )
        if flags[1]:
            m["vb"] = np.ascontiguousarray(bv[:, hh].reshape(L, 1, 256))
        if flags[2]:
            m["b1r"] = np.ascontiguousarray(bias1[:, None, :])
        if flags[3]:
            m["g1r"] = np.ascontiguousarray(ln1_g[:, None, :])
        if flags[4]:
            m["b1f"] = np.ascontiguousarray(b1f.reshape(L, FT, 128).transpose(0, 2, 1))
        if flags[5]:
            m["b2r"] = np.ascontiguousarray(bias2[:, None, :])
        if flags[6]:
            m["g2r"] = np.ascontiguousarray(ln2_g[:, None, :])
        if flags[7]:
            m["blmv"] = np.ascontiguousarray(blm_eff[None, VS * r:VS * (r + 1)])
        in_maps.append(m)

    r_ = run_bass_kernel_spmd(nc, in_maps, list(range(NC)))
    res = r_.results

    logits = np.empty((B * T, V), dtype=np.float32)
    Z = np.zeros((B, T), dtype=np.float64)
    for c in range(NC):
        b, r = divmod(c, TPG)
        logits[b * T:(b + 1) * T, VS * r:VS * (r + 1)] = res[c]["logits"]
        Z[b] += res[c]["zsum"][:, 0].astype(np.float64)
    lse = np.log(Z).reshape(B * T)
    tgt = targets.reshape(-1)
    tgt_logit = logits[np.arange(B * T), tgt].astype(np.float64)
    loss = np.float32(np.mean(lse - tgt_logit))
    return logits, loss
